# revision 1
# baseline (speedup 1.0000x reference)
"""Two-layer GCN (MultiOrderGraphLayer) Bass kernel for 8 Trainium2 cores.

Math: out = 0.5*(relu(A_hat@x@W1+b1) + relu(A_hat@x@W2+b2)) with
A_hat = D^-1/2 (A+I) D^-1/2.  Since both layers share A_hat, we compute
g = A_hat @ x once and apply the two small 128x128 matmuls afterwards.

Device algorithm (per core, feature-major layout g_T = [128 feat, nodes]):
  - nodes sharded 8 ways by row; edges partitioned by destination core.
  - self-loops are appended as ordinary edges with weight dinv[d]^2; every
    edge e carries norm_e = dinv[src]*dinv[dst] = sqrt(1/(deg[s]*deg[d])).
  - edges grouped per 128-node output window, padded to 128-edge blocks.
  - per block: dma_gather 128 rows of x (512B each); build the scaled
    one-hot S[e, n] = norm_e * (dstloc_e == n) in ONE fused DVE
    tensor_scalar (iota is_equal dstloc) mult dinv; accumulate
    t_T += xg^T @ S in PSUM over the window's blocks.
  - dma_gather indices are int16, so sources are split into lo (<32768)
    and hi (>=32768) streams; each window is accumulated in two phases.
  - finish: out[n, fo] = relu(g_T^T @ (0.5*W) + 0.5*b) summed over layers,
    written node-major straight from PSUM-shaped matmuls (no transposes).

Host-side prep is integer index manipulation only (partition, sort, pad,
degree counting); all float math runs on device.
"""

import math
import numpy as np

N_NODES = 50000
D = 128
N_CORES = 8
SPLIT = 32768  # int16 gather index limit
WIN = 128      # output-window size in nodes (one-hot width / psum free dim)
CHUNK = 4096   # indices per dma_gather instruction (multiple of 128)
N_QUEUES = 4   # SWDGE queues; rotating queue_num 4x's gather bandwidth
GF = 8         # one-hot blocks fused per DVE tensor_tensor op
X_BF16 = False  # bf16 gather path: ~10% faster, rel err 2e-3 vs 1e-6; keep f32


# ---------------------------------------------------------------- host prep

def host_prep(edge_index, n_nodes, n_cores, split=SPLIT, chunk=CHUNK):
    """Integer-only preprocessing: edge partitioning by destination,
    window grouping, lo/hi source split, padding, degree products.

    Returns (meta, per_core_inputs) where per_core_inputs[c] is a dict of
    numpy arrays for core c's DRAM parameters (excluding x/W/b/iota).
    """
    src = np.asarray(edge_index[0], dtype=np.int64)
    dst = np.asarray(edge_index[1], dtype=np.int64)
    deg = np.bincount(dst, minlength=n_nodes).astype(np.int64) + 1

    loop = np.arange(n_nodes, dtype=np.int64)
    s_all = np.concatenate([src, loop])
    d_all = np.concatenate([dst, loop])
    npr_all = (deg[s_all] * deg[d_all]).astype(np.float32)  # exact ints in f32

    npc = n_nodes // n_cores
    assert npc * n_cores == n_nodes
    nwin = math.ceil(npc / WIN)
    n_halves = 2 if n_nodes > split else 1

    per_core_sorted = []
    counts = np.zeros((n_cores, n_halves, nwin), np.int64)
    for c in range(n_cores):
        n0 = c * npc
        m = (d_all >= n0) & (d_all < n0 + npc)
        s, d, pr = s_all[m], d_all[m], npr_all[m]
        w = (d - n0) // WIN
        half = (s >= split).astype(np.int64) if n_halves == 2 else np.zeros_like(s)
        key = half * nwin + w
        order = np.argsort(key, kind="stable")
        s, d, pr, key = s[order], d[order], pr[order], key[order]
        cnt = np.bincount(key, minlength=n_halves * nwin)
        counts[c] = cnt.reshape(n_halves, nwin)
        per_core_sorted.append((s, d, pr, cnt))

    # shared block structure: blocks per (half, window), equal across cores
    nblk = np.maximum(1, -(-counts.max(axis=0) // 128))  # [n_halves, nwin]
    half_tot = nblk.sum(axis=1) * 128                    # edge slots per half
    nblk_tot = int(nblk.sum())

    # chunk split per half (shared across cores); small warmup chunks so
    # the first PSUM windows start within ~15us instead of waiting for a
    # full 2MB gather to drain through the round-robin SDMA queues
    chunk_sizes = []
    for h in range(n_halves):
        rem, sizes = int(half_tot[h]), []
        for warm in (1024, 1024, 2048, 2048):
            L = min(warm, rem)
            if L > 0:
                sizes.append(L)
                rem -= L
        while rem > 0:
            L = min(chunk, rem)
            sizes.append(L)
            rem -= L
        chunk_sizes.append(sizes)

    per_core_inputs = []
    for c in range(n_cores):
        s, d, pr, cnt = per_core_sorted[c]
        offs = np.concatenate([[0], np.cumsum(cnt)])
        idx_h = [[] for _ in range(n_halves)]
        dl_parts, pr_parts = [], []
        for h in range(n_halves):
            for wi in range(nwin):
                k = h * nwin + wi
                a, b = int(offs[k]), int(offs[k + 1])
                L = int(nblk[h, wi]) * 128
                pad = L - (b - a)
                gs = np.concatenate([s[a:b] - h * split,
                                     np.zeros(pad, np.int64)])
                gd = np.concatenate([(d[a:b] - c * npc - wi * WIN).astype(np.float32),
                                     np.full(pad, -1.0, np.float32)])
                gp = np.concatenate([pr[a:b], np.ones(pad, np.float32)])
                idx_h[h].append(gs.astype(np.int16))
                dl_parts.append(gd)
                pr_parts.append(gp)

        # one-hot metadata, block-major -> [128 lanes, nblk_tot]
        dl_stream = np.concatenate(dl_parts).reshape(-1, 128)
        pr_stream = np.concatenate(pr_parts).reshape(-1, 128)
        core_in = {
            "dstloc": np.ascontiguousarray(dl_stream.T),
            "normprod": np.ascontiguousarray(pr_stream.T),
        }
        # gather indices: wrapped [16, L/16] per chunk, replicated to 128 rows
        for h in range(n_halves):
            stream = np.concatenate(idx_h[h])
            cols, off = [], 0
            for L in chunk_sizes[h]:
                a = stream[off:off + L].reshape(-1, 16).T  # [16, L/16]
                cols.append(a)
                off += L
            wrapped = np.concatenate(cols, axis=1)         # [16, half_tot/16]
            core_in["idx_h%d" % h] = np.ascontiguousarray(
                np.tile(wrapped, (8, 1)))
        per_core_inputs.append(core_in)

    meta = dict(n_nodes=n_nodes, n_cores=n_cores, npc=npc, nwin=nwin,
                n_halves=n_halves, split=split, nblk=nblk,
                half_tot=half_tot, nblk_tot=nblk_tot, chunk=chunk,
                chunk_sizes=chunk_sizes)
    return meta, per_core_inputs


# ------------------------------------------------------------- bass program

def build_program(meta):
    import concourse.bacc as bacc
    import concourse.mybir as mybir
    import concourse.tile as tile
    from concourse import library_config

    f32 = mybir.dt.float32
    i16 = mybir.dt.int16
    xdt = mybir.dt.bfloat16 if X_BF16 else f32
    AF = mybir.ActivationFunctionType
    OP = mybir.AluOpType

    n_nodes = meta["n_nodes"]
    npc, nwin = meta["npc"], meta["nwin"]
    n_halves, split = meta["n_halves"], meta["split"]
    nblk, nblk_tot = meta["nblk"], meta["nblk_tot"]
    chunk = meta["chunk"]
    chunk_sizes = meta["chunk_sizes"]

    nc = bacc.Bacc("TRN2", num_swdge_queues=N_QUEUES)

    x_d = nc.declare_dram_parameter("x", [n_nodes, D], xdt, isOutput=False)
    dl_d = nc.declare_dram_parameter("dstloc", [128, nblk_tot], f32, isOutput=False)
    pr_d = nc.declare_dram_parameter("normprod", [128, nblk_tot], f32, isOutput=False)
    idx_d = [nc.declare_dram_parameter("idx_h%d" % h,
                                       [128, int(meta["half_tot"][h]) // 16],
                                       i16, isOutput=False)
             for h in range(n_halves)]
    w1_d = nc.declare_dram_parameter("W1", [D, D], f32, isOutput=False)
    w2_d = nc.declare_dram_parameter("W2", [D, D], f32, isOutput=False)
    b1_d = nc.declare_dram_parameter("b1", [1, D], f32, isOutput=False)
    b2_d = nc.declare_dram_parameter("b2", [1, D], f32, isOutput=False)
    iota_d = nc.declare_dram_parameter("iota", [128, GF * 128], f32,
                                       isOutput=False)
    out_d = nc.declare_dram_parameter("out", [npc, D], f32, isOutput=True)

    WG = 4  # windows per phase-2 batch (one 512-wide psum bank)

    with tile.TileContext(nc) as tc:
        with (
            tc.tile_pool(name="const", bufs=1) as constp,
            tc.tile_pool(name="xg", bufs=6) as xgp,
            tc.tile_pool(name="eq", bufs=3) as eqp,
            tc.tile_pool(name="oh", bufs=4) as ohp,
            tc.tile_pool(name="ps1", bufs=3, space="PSUM") as ps1,
            tc.tile_pool(name="ps2", bufs=2, space="PSUM") as ps2,
            tc.tile_pool(name="fin", bufs=3) as finp,
        ):
            # Q7 library holding DMAGatherAnt; must precede all gathers
            nc.gpsimd.load_library(library_config.mlp)

            # --- constants / metadata
            iota8 = constp.tile([128, GF, 128], f32)
            nc.sync.dma_start(
                iota8[:], iota_d[:].rearrange("p (c n) -> p c n", n=128))
            wts = {}
            for nm, src_d in (("w1", w1_d), ("w2", w2_d)):
                raw = constp.tile([128, 128], f32, tag=nm + "raw")
                nc.sync.dma_start(raw[:], src_d[:])
                half = constp.tile([128, 128], f32, tag=nm + "half")
                nc.scalar.activation(half[:], raw[:], AF.Copy, scale=0.5)
                wts[nm] = half
            bias = {}
            for nm, src_d in (("b1", b1_d), ("b2", b2_d)):
                raw = constp.tile([1, 128], f32, tag=nm + "raw")
                nc.sync.dma_start(raw[:], src_d[:])
                half = constp.tile([1, 128], f32, tag=nm + "half")
                nc.scalar.activation(half[:], raw[:], AF.Copy, scale=0.5)
                bias[nm] = half
            ones = constp.tile([1, 128], f32)
            nc.vector.memset(ones[:], 1.0)

            dl = constp.tile([128, nblk_tot], f32)
            nc.sync.dma_start(dl[:], dl_d[:])
            pr = constp.tile([128, nblk_tot], f32)
            nc.sync.dma_start(pr[:], pr_d[:])
            rec = constp.tile([128, nblk_tot], f32)
            nc.vector.reciprocal(rec[:], pr[:])
            dinv = constp.tile([128, nblk_tot], f32)
            nc.scalar.activation(dinv[:], rec[:], AF.Sqrt)

            g_all = constp.tile([128, npc], f32)

            # idx streams fully preloaded (tiny); dma_gather slices them
            idx_all = []
            for h in range(n_halves):
                t = constp.tile([128, int(meta["half_tot"][h]) // 16], i16,
                                tag="idx%d" % h)
                nc.sync.dma_start(t[:], idx_d[h][:])
                idx_all.append(t)

            # one-hot groups: GF blocks fused per DVE op; one active
            # group cached per half (streams are consumed interleaved)
            oh_cache = {}

            def get_oh(bg, h):
                g = bg // GF
                if oh_cache.get(h, (None, None))[0] != g:
                    g0 = g * GF
                    gl = min(GF, nblk_tot - g0)
                    eq = eqp.tile([128, GF, 128], xdt, tag="eq")
                    nc.vector.tensor_tensor(
                        out=eq[:, :gl, :], in0=iota8[:, :gl, :],
                        in1=dl[:, g0:g0 + gl, None].to_broadcast([128, gl, 128]),
                        op=OP.is_equal)
                    oh = ohp.tile([128, GF, 128], xdt, tag="oh")
                    nc.vector.tensor_tensor(
                        out=oh[:, :gl, :], in0=eq[:, :gl, :],
                        in1=dinv[:, g0:g0 + gl, None].to_broadcast([128, gl, 128]),
                        op=OP.mult)
                    oh_cache[h] = (g, oh)
                return oh_cache[h][1]

            # per-half stream state: lazy chunk issuing in window order
            class Stream:
                pass

            streams = []
            blk_base = 0
            for h in range(n_halves):
                s = Stream()
                s.h = h
                s.base = x_d[0:split, :] if h == 0 else x_d[split:n_nodes, :]
                s.wstart = np.concatenate([[0], np.cumsum(nblk[h])])
                s.blk_base = blk_base          # global block id of stream pos 0
                s.chunks = []                  # (off, L) already issued -> tile
                s.chunk_bounds = []
                off = 0
                for L in chunk_sizes[h]:
                    s.chunk_bounds.append((off, L))
                    off += L
                s.blk2chunk = np.repeat(
                    np.arange(len(chunk_sizes[h])),
                    [L // 128 for L in chunk_sizes[h]])
                s.next_chunk = 0
                s.tiles = {}
                blk_base += int(nblk[h].sum())
                streams.append(s)

            ci_global = 0

            def ensure_chunk(s, ci):
                nonlocal ci_global
                if ci in s.tiles:
                    return s.tiles[ci]
                off, L = s.chunk_bounds[ci]
                xg = xgp.tile([128, chunk // 128, 128], xdt, tag="xg")
                nc.gpsimd.dma_gather(
                    out_ap=xg[:, : L // 128, :],
                    in_ap=s.base,
                    idxs_ap=idx_all[s.h][:, off // 16:(off + L) // 16],
                    num_idxs=L,
                    num_idxs_reg=L,
                    elem_size=D,
                    single_packet=False,
                    queue_num=ci_global % N_QUEUES,
                )
                ci_global += 1
                s.tiles.clear()
                s.tiles[ci] = xg
                return xg

            # --- fused pass: per window accumulate lo+hi edge blocks in
            # one PSUM group, flush to g_all; every WG windows run the
            # output stage (overlaps with later windows' aggregation).
            def emit_phase2(wlo, whi):
                nwg = whi - wlo + 1
                wls = [min(WIN, npc - w * WIN) for w in range(wlo, whi + 1)]
                rows = min(wls)  # < 128 only for a trailing ragged window
                pps = {}
                for nm_w, nm_b in (("w1", "b1"), ("w2", "b2")):
                    pp = ps2.tile([128, WG * 128], f32, tag="pp")
                    for j, w in enumerate(range(wlo, whi + 1)):
                        wl = wls[j]
                        sl = pp[:wl, j * 128:(j + 1) * 128]
                        nc.tensor.matmul(sl, g_all[:, w * WIN:w * WIN + wl],
                                         wts[nm_w][:], start=True, stop=False)
                        nc.tensor.matmul(sl, ones[:, :wl], bias[nm_b][:],
                                         start=False, stop=True)
                    o = finp.tile([128, WG, 128], f32, tag="o" + nm_w)
                    of = o[:].rearrange("p c n -> p (c n)")
                    if rows == 128:
                        nc.scalar.activation(of[:, :nwg * 128],
                                             pp[:, :nwg * 128], AF.Relu)
                    else:
                        for j in range(nwg):
                            nc.scalar.activation(
                                of[:wls[j], j * 128:(j + 1) * 128],
                                pp[:wls[j], j * 128:(j + 1) * 128], AF.Relu)
                    pps[nm_w] = o
                ot = finp.tile([128, WG, 128], f32, tag="ot")
                otf = ot[:].rearrange("p c n -> p (c n)")
                o1f = pps["w1"][:].rearrange("p c n -> p (c n)")
                o2f = pps["w2"][:].rearrange("p c n -> p (c n)")
                if rows == 128:
                    nc.vector.tensor_tensor(otf[:, :nwg * 128],
                                            o1f[:, :nwg * 128],
                                            o2f[:, :nwg * 128], op=OP.add)
                else:
                    for j in range(nwg):
                        cs = slice(j * 128, j * 128 + 128)
                        nc.vector.tensor_tensor(otf[:wls[j], cs],
                                                o1f[:wls[j], cs],
                                                o2f[:wls[j], cs], op=OP.add)
                for j, w in enumerate(range(wlo, whi + 1)):
                    nc.sync.dma_start(out_d[w * WIN:w * WIN + wls[j], :],
                                      ot[:wls[j], j, :])

            for w in range(nwin):
                wlen = min(WIN, npc - w * WIN)
                pw = ps1.tile([128, 128], f32, tag="pw")
                # total blocks this window across halves
                runs = []
                for s in streams:
                    b0, b1 = int(s.wstart[w]), int(s.wstart[w + 1])
                    runs.append((s, b0, b1))
                n_tot = sum(b1 - b0 for _, b0, b1 in runs)
                k = 0
                for s, b0, b1 in runs:
                    for b in range(b0, b1):
                        ci = int(s.blk2chunk[b])
                        xg = ensure_chunk(s, ci)
                        bl = (b * 128 - s.chunk_bounds[ci][0]) // 128
                        bg = s.blk_base + b
                        oh = get_oh(bg, s.h)
                        nc.tensor.matmul(
                            pw[:, :wlen],
                            xg[:, bl, :],
                            oh[:, bg % GF, :wlen],
                            start=(k == 0),
                            stop=(k == n_tot - 1),
                        )
                        k += 1
                nc.scalar.activation(g_all[:, w * WIN:w * WIN + wlen],
                                     pw[:, :wlen], AF.Copy)
                if w % WG == WG - 1 or w == nwin - 1:
                    emit_phase2(w - (w % WG), w)

    nc.compile()
    return nc


def make_core_inputs(meta, per_core_inputs, x, W1, b1, W2, b2):
    """Full in_maps for run_bass_kernel_spmd (adds shared tensors)."""
    if X_BF16:
        import ml_dtypes
        x = np.ascontiguousarray(np.asarray(x).astype(ml_dtypes.bfloat16))
    else:
        x = np.ascontiguousarray(np.asarray(x, dtype=np.float32))
    shared = {
        "x": x,
        "W1": np.ascontiguousarray(np.asarray(W1, np.float32)),
        "W2": np.ascontiguousarray(np.asarray(W2, np.float32)),
        "b1": np.asarray(b1, np.float32).reshape(1, D),
        "b2": np.asarray(b2, np.float32).reshape(1, D),
        "iota": np.ascontiguousarray(np.broadcast_to(
            np.tile(np.arange(128, dtype=np.float32), GF), (128, GF * 128))),
    }
    return [dict(shared, **ci) for ci in per_core_inputs]


# ------------------------------------------------------------------- kernel

def kernel(x, edge_index, W1, b1, W2, b2, _trace=False):
    from concourse.bass_utils import run_bass_kernel_spmd

    x = np.asarray(x)
    n_nodes = x.shape[0]
    meta, pci = host_prep(edge_index, n_nodes, N_CORES)
    nc = build_program(meta)
    in_maps = make_core_inputs(meta, pci, x, W1, b1, W2, b2)
    res = run_bass_kernel_spmd(nc, in_maps, list(range(N_CORES)),
                               trace=_trace)
    out = np.concatenate([res.results[c]["out"] for c in range(N_CORES)],
                         axis=0)
    if _trace:
        return out, res
    return out

